# revision 45
# baseline (speedup 1.0000x reference)
"""Trainium2 Bass kernel for nn_Block_59983513256170 (dense transformer block).

Sharding: 8 cores = (batch 4) x (sequence halves 2). Each core computes the
block for 512 query tokens of one batch element, redundantly computing
LN1+quant and K/V over that batch element's full 1024-token sequence so no
cross-core communication is needed. Host rotates each core's token order so
its own 512 tokens are always rows [0:512] (attention is permutation
invariant over keys), letting all cores run one identical SPMD program.

Precision: attention branch (qkv/scores/AV/proj) runs on fp32r matmuls
(~12 mantissa bits at bf16 speed) because its output feeds the second
block-FP quantization, where noise flips quantization bins. The quantized
h1/h2 values (4-bit mantissa) are exact in bf16/fp8: transposes run through
the DMA XBAR in bf16, h2 feeds fc1 as fp8 (DoubleRow), fc2 runs in bf16.
LN, BFP quant, softmax arithmetic and residuals are fp32.

Schedule highlights (all engine-balanced + software-pipelined):
 - LN1: bn-stats of tile t+1 emitted before tile t's apply+quant (DVE never
   ping-pongs on ACT); chain split DVE/GpSimd/ACT; token->feature
   transposes via DMA XBAR (bf16 staging ring + ACT convert to f32r);
   v-units per tile and q/k units fill the PE during the LN tail.
 - Attention: the two heads of a pair run as concurrent row-tiled
   64-partition QK matmuls into one 2-bank psum, exp is fused over both
   banks in a single ACTIVATE, AV trails QK by one key chunk, and
   background PE work (remaining k/v units, then proj partial sums for
   heads 0-7) fills the exp-latency gaps so HAM stays at full clock.
 - Softmax normalization: fused psum*recip multiply (scalar_tensor_tensor)
   with a gpsimd partition-broadcast of the reciprocal row.
 - Back half: proj finishes (heads 8-15) skewed with LN2; fc1 (fp8
   DoubleRow) runs in token halves, the first bridging the LN2 tail so the
   PE clock never drops; fc2 shares one stationary load per output pair;
   residual adds on DVE.
"""

import sys

sys.path.insert(0, "/opt/trn_rl_repo")

import numpy as np
import ml_dtypes

import concourse.bass as bass
import concourse.bacc as bacc
import concourse.tile as tile
import concourse.mybir as mybir
from concourse import bass_utils

F32 = mybir.dt.float32
F32R = mybir.dt.float32r
BF16 = mybir.dt.bfloat16
F8 = mybir.dt.float8e4
DRM = mybir.MatmulPerfMode.DoubleRow
AF = mybir.ActivationFunctionType
OP = mybir.AluOpType

D = 1024
H = 16
DH = 64
DFF = 4096
LN_EPS = 1e-6


def bcast16(t):
    """View a [128, nb] tile as [128, nb, 16] with the last dim broadcast."""
    ap = [list(x) for x in t.ap]
    return bass.AP(tensor=t.tensor, offset=t.offset, ap=ap + [[0, 16]])


def pbcast(t, n):
    """View a [1, ...] tile as [n, ...] with the partition dim broadcast
    (for DMA replication)."""
    ap = [list(x) for x in t.ap]
    return bass.AP(tensor=t.tensor, offset=t.offset, ap=[[0, n]] + ap[1:])


def build_nc(Tq, Tkv, apply_gb=True, has_bp=False, has_b2=False,
             has_b1=False):
    """Build the per-core Bass program. Tq = own query tokens, Tkv = full
    sequence tokens of this core's batch element (own tokens first)."""
    nq = Tq // 128   # query token tiles (4)
    nk = Tkv // 128  # kv token tiles (8)
    npair = H // 2   # head pairs (8)
    nc = bacc.Bacc("TRN2", target_bir_lowering=False, debug=False)

    x_d = nc.dram_tensor("x", [Tkv, D], F32, kind="ExternalInput").ap()
    wqkv_d = nc.dram_tensor("w_qkv", [D, 3 * D], F32R, kind="ExternalInput").ap()
    wproj_d = nc.dram_tensor("w_proj", [D, D], F32R, kind="ExternalInput").ap()
    bproj_d = nc.dram_tensor("b_proj", [D], F32, kind="ExternalInput").ap()
    wfc1_d = nc.dram_tensor("w_fc1", [DFF // 128, 128, 4, 2, 128], F8,
                            kind="ExternalInput").ap()
    bfc1_d = nc.dram_tensor("b_fc1", [DFF], F32, kind="ExternalInput").ap()
    wfc2_d = nc.dram_tensor("w_fc2", [DFF, D], BF16, kind="ExternalInput").ap()
    bfc2_d = nc.dram_tensor("b_fc2", [D], F32, kind="ExternalInput").ap()
    g1_d = nc.dram_tensor("ln1_g", [D], F32, kind="ExternalInput").ap()
    b1_d = nc.dram_tensor("ln1_b", [D], F32, kind="ExternalInput").ap()
    g2_d = nc.dram_tensor("ln2_g", [D], F32, kind="ExternalInput").ap()
    b2_d = nc.dram_tensor("ln2_b", [D], F32, kind="ExternalInput").ap()
    out_d = nc.dram_tensor("out", [Tq, D], F32, kind="ExternalOutput").ap()

    def vec_bcast(pool, dram_vec, name, dtype=F32):
        """DRAM [D] vector -> SBUF [128, D] broadcast tile."""
        t = pool.tile([128, dram_vec.shape[0]], dtype, name=name)
        src = bass.AP(tensor=dram_vec.tensor, offset=dram_vec.offset,
                      ap=[[0, 128]] + [list(x) for x in dram_vec.ap])
        eng = nc.gpsimd if dtype != dram_vec.dtype else nc.sync
        eng.dma_start(out=t, in_=src)
        return t

    with tile.TileContext(nc) as tc:
        _cms = {}

        def open_pool(name, bufs, space="SBUF", side="left"):
            cm = tc.tile_pool(name=name, bufs=bufs, space=space, side=side)
            _cms[name] = cm
            return cm.__enter__()

        def close_pool(name):
            _cms.pop(name).__exit__(None, None, None)

        # non-default variants carry extra broadcast consts; shrink rings
        tight = apply_gb or has_bp or has_b2 or has_b1
        # ---- pool stacks (LIFO per side) ----
        consts = open_pool("consts", 1)
        small = open_pool("small", 2)
        resid = open_pool("resid", 1)
        attn_big = open_pool("attn_big", 1)
        h1fmp = open_pool("h1fmp", 1)
        wk_pool = open_pool("wk", 1)
        wv_pool = open_pool("wv", 1)
        h1t_pool = open_pool("h1t", 2 if tight else 3)
        lnx = open_pool("lnx", 1 if tight else 2)  # x tiles 4..7
        lnh = open_pool("lnh", 2)  # LN ht scratch
        ln_q = open_pool("ln_q", 1 if tight else 2)
        wq_pool = open_pool("wq", 1 if tight else 2)
        # psum
        psA = open_pool("psA", 2, space="PSUM")
        warm_ps = open_pool("warm_ps", 1, space="PSUM")

        eps_t = consts.tile([128, 1], F32, name="eps")
        nc.vector.memset(eps_t, LN_EPS)
        idw = consts.tile([128, 128], BF16, name="idw")
        nc.vector.memset(idw, 0.0)
        if not tight:
            idw512 = consts.tile([128, 512], BF16, name="idw512")
            nc.vector.memset(idw512, 0.0)
        else:
            idw512 = idw
        cdt = BF16 if tight else F32
        if apply_gb:
            g1b = vec_bcast(consts, g1_d, "g1b", cdt)
            b1b = vec_bcast(consts, b1_d, "b1b", cdt)
            g2b = vec_bcast(consts, g2_d, "g2b", cdt)
            b2b = vec_bcast(consts, b2_d, "b2b", cdt)
        else:
            g1b = b1b = g2b = b2b = None
        bpb = vec_bcast(consts, bproj_d, "bpb", cdt) if has_bp else None
        bf2b = vec_bcast(consts, bfc2_d, "bf2b", cdt) if has_b2 else None
        # b_fc1 as per-partition bias columns: [128, 32], [p, c] = b_fc1[c*128+p]
        bfc1_sb = consts.tile([128, DFF // 128], F32, name="bfc1")
        nc.sync.dma_start(out=bfc1_sb, in_=bfc1_d.rearrange("(c p) -> p c", p=128))

        def warm(n=64, pool=None, reps=1):
            """Small PE touch to keep/raise the HAM clock during PE-sparse
            stretches."""
            n = min(n, 128 if tight else 512)
            dp = (pool or warm_ps).tile([128, n], F32,
                                        name="ps" if pool else "warm")
            for r in range(reps):
                nc.tensor.matmul(dp, idw, idw512[:, 0:n],
                                 start=True, stop=True)

        # persistent tiles
        h1_fm = h1fmp.tile([128, 8, Tkv], F32R, name="h1fm")  # feature-major
        qT = attn_big.tile([128, npair, Tq], F32R, name="qT")
        kT = attn_big.tile([128, npair, Tkv], F32R, name="kT")
        v65 = attn_big.tile([128, H, nk, 65], F32R, name="v65")
        x2 = resid.tile([128, nq, D], F32, name="x2")
        # x2 doubles as the LN1 input for the own-token tiles 0..3
        for t in range(nq):
            nc.sync.dma_start(out=x2[:, t, :], in_=x_d[t * 128:(t + 1) * 128, :])

        # v65 ones plane (denominator column), one strided memset
        nc.gpsimd.memset(v65[:, :, :, 64:65].bitcast(F32), 1.0)

        def ln_seg1(xt, name="h"):
            """bn stats + sqrt: DVE then ACT, no downstream deps yet."""
            st = small.tile([128, 2, 6], F32, name=name + "st")
            nc.vector.bn_stats(out=st[:, 0, :], in_=xt[:, 0:512])
            nc.vector.bn_stats(out=st[:, 1, :], in_=xt[:, 512:1024])
            mv = small.tile([128, 2], F32, name=name + "mv")
            nc.vector.bn_aggr(out=mv, in_=st)
            rs = small.tile([128, 1], F32, name=name + "rs")
            nc.scalar.activation(rs, mv[:, 1:2], AF.Sqrt, bias=eps_t, scale=1.0)
            return xt, mv, rs

        def ln_seg2(seg, g_b, b_b, ht_pool, hq_out_pool, name="h"):
            """recip + apply (ACT Identity) + BFP quant (DVE/GpSimd)."""
            xt, mv, rs = seg
            rr = small.tile([128, 1], F32, name=name + "rr")
            nc.vector.reciprocal(rr, rs)
            nmr = small.tile([128, 1], F32, name=name + "nmr")
            nc.vector.tensor_tensor(out=nmr, in0=mv[:, 0:1], in1=rr, op=OP.mult)
            nc.vector.tensor_scalar_mul(nmr, nmr, -1.0)
            ht = ht_pool.tile([128, D], F32, name="lnb")
            nc.scalar.activation(ht, xt, AF.Identity, bias=nmr, scale=rr)
            if g_b is not None:
                nc.gpsimd.tensor_tensor(out=ht, in0=ht, in1=g_b, op=OP.mult)
                nc.gpsimd.tensor_tensor(out=ht, in0=ht, in1=b_b, op=OP.add)
            nb = D // 16
            amax = small.tile([128, nb], F32, name=name + "amax")
            nc.vector.tensor_reduce(amax, ht.rearrange("p (b k) -> p b k", k=16),
                                    axis=mybir.AxisListType.X, op=OP.max,
                                    apply_absolute_value=True)
            nc.vector.tensor_scalar(out=amax.bitcast(mybir.dt.uint32),
                                    in0=amax.bitcast(mybir.dt.uint32),
                                    scalar1=0xFF800000, scalar2=None,
                                    op0=OP.bitwise_and)
            cc = small.tile([128, nb], F32, name=name + "cc")
            nc.vector.tensor_scalar_mul(cc, amax, float(1.5 * 2 ** 20))
            hi = small.tile([128, nb], F32, name=name + "hi")
            nc.vector.tensor_scalar_mul(hi, amax, 7.0 / 8.0)
            lo = small.tile([128, nb], F32, name=name + "lo")
            nc.vector.tensor_scalar_mul(lo, amax, -7.0 / 8.0)
            h3 = ht.rearrange("p (b k) -> p b k", k=16)
            nc.gpsimd.tensor_tensor(out=h3, in0=h3, in1=bcast16(cc), op=OP.add)
            nc.gpsimd.tensor_tensor(out=h3, in0=h3, in1=bcast16(cc),
                                    op=OP.subtract)
            nc.vector.tensor_tensor(out=h3, in0=h3, in1=bcast16(hi), op=OP.min)
            hq = hq_out_pool.tile([128, D], BF16, name=name + "q")
            nc.vector.tensor_tensor(out=hq.rearrange("p (b k) -> p b k", k=16),
                                    in0=h3, in1=bcast16(lo), op=OP.max)
            return hq

        def layernorm_quant(xt, g_b, b_b, ht_pool, hq_out_pool, name="h"):
            return ln_seg2(ln_seg1(xt, name), g_b, b_b, ht_pool, hq_out_pool,
                           name)

        # ---- qkv unit emitters ----
        def q_unit(j, wq):
            """q for head pair j: psum bank <- 8 accum MMs, copy to qT (ACT)."""
            ps = psA.tile([128, Tq], F32, name="ps")
            for d in range(8):
                nc.tensor.matmul(ps, wq[:, d, :], h1_fm[:, d, 0:Tq],
                                 start=(d == 0), stop=(d == 7))
            nc.scalar.copy(qT[:, j, :], ps)

        def load_wk(j0, engs=None):
            wk = wk_pool.tile([128, 4, 8, 128], F32R, name="wk")
            for j in range(j0, j0 + 4):
                eng = (engs or (nc.gpsimd,))[j % len(engs or (nc.gpsimd,))]
                eng.dma_start(
                    out=wk[:, j - j0, :, :],
                    in_=wqkv_d[:, D + j * 128:D + (j + 1) * 128]
                    .rearrange("(a p) c -> p a c", p=128))
            return wk

        wk_state = {}

        def k_unit(j, th):
            """k for head pair j, token half th: 8 accum MMs + DVE copy."""
            wk = wk_state[j // 4]
            ps = psA.tile([128, 512], F32, name="ps")
            for d in range(8):
                nc.tensor.matmul(ps, wk[:, j % 4, d, :],
                                 h1_fm[:, d, th * 512:(th + 1) * 512],
                                 start=(d == 0), stop=(d == 7))
            nc.vector.tensor_copy(out=kT[:, j, th * 512:(th + 1) * 512], in_=ps)

        def v_unit(vc, tch, wv, copy_eng=None):
            """v for head half vc, token tile tch: 8 accum MMs + one copy
            into v65 (8 heads' 64-col slices)."""
            ps = psA.tile([128, 512], F32, name="ps")
            for d in range(8):
                nc.tensor.matmul(ps, h1_fm[:, d, tch * 128:(tch + 1) * 128],
                                 wv[:, d, :], start=(d == 0), stop=(d == 7))
            dst = v65[:, vc * 8:(vc + 1) * 8, tch, 0:64]
            srcp = ps.rearrange("p (h e) -> p h e", e=64)
            if copy_eng is nc.scalar:
                nc.scalar.copy(dst, srcp)
            else:
                nc.vector.tensor_copy(out=dst, in_=srcp)

        # ---------------- phase A: LN1 + qkv interleaved ----------------
        wq_tiles = {}
        with nc.named_scope("ln1_qkv"):
            wv0 = wv_pool.tile([128, 8, 512], F32R, name="wv")
            for d in range(8):
                nc.gpsimd.dma_start(out=wv0[:, d, :],
                                    in_=wqkv_d[d * 128:(d + 1) * 128, 2048:2560])
            wk_state[0] = load_wk(0)

            def load_wq(j):
                wq = wq_pool.tile([128, 8, 128], F32R, name="wq")
                nc.sync.dma_start(
                    out=wq,
                    in_=wqkv_d[:, j * 128:(j + 1) * 128]
                    .rearrange("(a p) c -> p a c", p=128))
                wq_tiles[j] = wq

            # preload x tiles 4..7 (ring 2: 6/7 issued mid-loop)
            xts = {}
            for t in (4, 5):
                xts[t] = lnx.tile([128, D], F32, name="xt")
                nc.sync.dma_start(out=xts[t],
                                  in_=x_d[t * 128:(t + 1) * 128, :])
            segs = {0: ln_seg1(x2[:, 0, :], name="h1")}
            for t in range(nk):
                # stats of the next tile first: keeps DVE fed while ACT/GpSimd
                # work on this tile's apply+quant
                if t + 1 < nk:
                    if t + 1 < nq:
                        xt1 = x2[:, t + 1, :]
                    else:
                        xt1 = xts.pop(t + 1)
                    segs[t + 1] = ln_seg1(xt1, name="h1")
                hq = ln_seg2(segs.pop(t), g1b, b1b, lnh, ln_q, name="h1")
                tr = h1t_pool.tile([128, 8, 128], BF16, name="h1t")
                nc.scalar.dma_start_transpose(out=tr, in_=hq)
                nc.scalar.copy(h1_fm[:, :, t * 128:(t + 1) * 128], tr)
                warm()
                v_unit(0, t, wv0, copy_eng=nc.scalar)
                if t == 3:
                    load_wq(0)
                    load_wq(1)
                elif t in (4, 5, 6):
                    if t in (4, 5):
                        xts[t + 2] = lnx.tile([128, D], F32, name="xt")
                        nc.sync.dma_start(out=xts[t + 2],
                                          in_=x_d[(t + 2) * 128:(t + 3) * 128, :])
                    j0 = (t - 4) * 2
                    for j in (j0, j0 + 1):
                        q_unit(j, wq_tiles.pop(j))
                        if j + 2 < npair:
                            load_wq(j + 2)
                elif t == 7:
                    for j in (6, 7):
                        q_unit(j, wq_tiles.pop(j))
                    for j in range(4):
                        k_unit(j, 0)
            # k(th1) for j=0,1 so attention can start
            k_unit(0, 1)
            k_unit(1, 1)
        close_pool("wq")
        close_pool("ln_q")
        close_pool("lnh")
        close_pool("lnx")
        close_pool("h1t")
        close_pool("warm_ps")

        if has_bp:
            for tcq in range(nq):
                nc.vector.tensor_tensor(out=x2[:, tcq, :], in0=x2[:, tcq, :],
                                        in1=bpb, op=OP.add)

        # ---------------- attention + background work ----------------
        ps_pair = open_pool("ps_pair", 2, space="PSUM")  # [128,2,512] F32
        ps_o = open_pool("ps_o", 1, space="PSUM")        # [65,2,512] F32
        o_pool = open_pool("o_pool", 1, side="right")
        ee_pool = open_pool("ee_pool", 2 if tight else 3, side="right")
        nrm = open_pool("nrm", 1, side="right")
        o_p = o_pool.tile([128, npair, Tq], F32R, name="o_p")
        wproj_sb = [None, None]
        wfc1_sb_box = [None]
        bg_state = {}

        def bg_k(j, th=1):
            def f():
                k_unit(j, th)
            return f

        def bg_wk1_load():
            def f():
                wk_state[1] = load_wk(4, engs=(nc.sync, nc.gpsimd))
            return f

        def bg_wv1_load():
            def f():
                wv1 = wv_pool.tile([128, 8, 512], F32R, name="wv")
                for d in range(8):
                    eng = nc.sync if d % 2 == 0 else nc.gpsimd
                    eng.dma_start(
                        out=wv1[:, d, :],
                        in_=wqkv_d[d * 128:(d + 1) * 128, 2560:3072])
                bg_state["wv1"] = wv1
            return f

        def bg_v1(tch):
            def f():
                v_unit(1, tch, bg_state["wv1"])
            return f

        def bg_close_a():
            def f():
                close_pool("wv")
                close_pool("wk")
                close_pool("h1fmp")
                wpp = open_pool("wprojA", 1)  # left, above attn_big
                wp = wpp.tile([128, 4, D], F32R, name="wprojA")
                for d in range(4):
                    eng = nc.sync if d % 2 == 0 else nc.gpsimd
                    eng.dma_start(out=wp[:, d, :],
                                  in_=wproj_d[d * 128:(d + 1) * 128, :])
                wproj_sb[0] = wp
            return f

        def proj_unit(tcq, nn, j0, j1):
            """Partial proj over head pairs [j0,j1) for (token tile, half),
            accumulated in psum then added into x2 on DVE."""
            def f():
                ps = psA.tile([128, 512], F32, name="ps")
                wp = wproj_sb[0] if j0 == 0 else wproj_sb[1]
                for j in range(j0, j1):
                    nc.tensor.matmul(ps, o_p[:, j, tcq * 128:(tcq + 1) * 128],
                                     wp[:, j - j0, nn * 512:(nn + 1) * 512],
                                     start=(j == j0), stop=(j == j1 - 1))
                nc.vector.tensor_tensor(
                    out=x2[:, tcq, nn * 512:(nn + 1) * 512], in0=ps,
                    in1=x2[:, tcq, nn * 512:(nn + 1) * 512], op=OP.add)
            return f

        bg_sched = {
            0: [bg_k(2), bg_k(3), bg_wk1_load(), bg_wv1_load()],
            1: [bg_k(4, 0), bg_k(4, 1), bg_v1(0), bg_v1(1)],
            2: [bg_k(5, 0), bg_k(5, 1), bg_v1(2), bg_v1(3)],
            3: [bg_k(6, 0), bg_k(6, 1), bg_v1(4), bg_v1(5)],
            4: [bg_k(7, 0), bg_k(7, 1), bg_v1(6), bg_v1(7)],
            5: [bg_close_a(), proj_unit(0, 0, 0, 4), proj_unit(0, 1, 0, 4),
                proj_unit(1, 0, 0, 4)],
            6: [proj_unit(1, 1, 0, 4), proj_unit(2, 0, 0, 4),
                proj_unit(2, 1, 0, 4)],
            7: [proj_unit(3, 0, 0, 4), proj_unit(3, 1, 0, 4)],
        }

        with nc.named_scope("attn"):
            for j in range(npair):
                units = list(bg_sched[j])
                pos = 0

                def pop_bg():
                    nonlocal pos
                    if pos < len(units):
                        units[pos]()
                        pos += 1
                    else:
                        warm(512, pool=psA, reps=2)

                po = ps_o.tile([65, 2, Tq], F32, name="pso")
                ees = {}
                for kc in range(nk):
                    pp = ps_pair.tile([128, 2, Tq], F32, name="pp")
                    for ab in range(2):
                        nc.tensor.matmul(
                            pp[:, ab, :],
                            kT[ab * 64:(ab + 1) * 64, j,
                               kc * 128:(kc + 1) * 128],
                            qT[ab * 64:(ab + 1) * 64, j, :],
                            start=True, stop=True)
                    if kc > 0:
                        ee = ees.pop(kc - 1)
                        for ab in range(2):
                            nc.tensor.matmul(po[:, ab, :],
                                             v65[:, 2 * j + ab, kc - 1, :],
                                             ee[:, ab, :],
                                             start=(kc - 1 == 0), stop=False)
                    ee = ee_pool.tile([128, 2, Tq], F32R, name="ee")
                    nc.scalar.activation(ee, pp, AF.Exp, scale=0.125)
                    ees[kc] = ee
                    if kc % 2 == 1:
                        pop_bg()
                ee = ees.pop(nk - 1)
                for ab in range(2):
                    nc.tensor.matmul(po[:, ab, :], v65[:, 2 * j + ab, nk - 1, :],
                                     ee[:, ab, :], start=False, stop=True)
                while pos < len(units):
                    units[pos]()
                    pos += 1
                # normalize: o_p = po[0:64] * (1/po[64]) broadcast
                row = nrm.tile([1, 2, Tq], F32, name="row")
                nc.vector.tensor_copy(out=row, in_=po[64:65, :, :])
                rrow = nrm.tile([1, 2, Tq], F32, name="rrow")
                nc.vector.reciprocal_approx_fast(rrow, row)
                r64 = nrm.tile([64, 2, Tq], F32, name="r64")
                nc.gpsimd.partition_broadcast(r64, rrow)
                for ab in range(2):
                    nc.vector.scalar_tensor_tensor(
                        out=o_p[ab * 64:(ab + 1) * 64, j, :],
                        in0=po[0:64, ab, :], scalar=1.0, in1=r64[:, ab, :],
                        op0=OP.mult, op1=OP.mult)

        close_pool("nrm")
        close_pool("ee_pool")
        close_pool("ps_o")
        close_pool("ps_pair")
        close_pool("wprojA")
        close_pool("attn_big")

        # ---------------- phase B: proj(B) + LN2 + MLP ----------------
        psB = open_pool("psB", 2, space="PSUM")
        # wproj rows for heads 8-15 (right side: closes before o_pool)
        wpbp = open_pool("wprojB", 1, side="right")
        wpb = wpbp.tile([128, 4, D], F32R, name="wprojB")
        for d in range(4):
            nc.sync.dma_start(out=wpb[:, d, :],
                              in_=wproj_d[(4 + d) * 128:(5 + d) * 128, :])
        wproj_sb[1] = wpb
        # fc1 weights (left): DMA starts now, consumed from fc1 onwards
        mlpw = open_pool("mlpw", 1)
        wfc1_sb = mlpw.tile([128, DFF // 128, 4, 2, 128], F8, name="wfc1")
        for hc in range(DFF // 128):
            eng = nc.sync if hc % 2 == 0 else nc.gpsimd
            eng.dma_start(out=wfc1_sb[:, hc], in_=wfc1_d[hc])
        mlp = open_pool("mlp", 1)
        mT = mlp.tile([128, DFF // 128, Tq], BF16, name="mT")
        h2bp = open_pool("h2bp", 1)
        h2Tb = h2bp.tile([128, 8, Tq], BF16, name="h2Tb")
        h2T = h2bp.tile([128, 8, Tq], F8, name="h2T")
        ln2buf = open_pool("ln2buf", 2)
        ln2_q = open_pool("ln2_q", 2)

        def fc1_half(h):
            for hc in range(DFF // 128):
                ps = psB.tile([128, 2, 512], F32, name="ps")
                for d2 in range(4):
                    nc.tensor.matmul(
                        ps[:, 0, 0:256], wfc1_sb[:, hc, d2, :, :],
                        h2T[:, 2 * d2:2 * d2 + 2, h * 256:(h + 1) * 256],
                        start=(d2 == 0), stop=(d2 == 3), perf_mode=DRM)
                if has_b1:
                    nc.scalar.activation(
                        mT[:, hc, h * 256:(h + 1) * 256], ps[:, 0, 0:256],
                        AF.Gelu, bias=bfc1_sb[:, hc:hc + 1], scale=1.0 / 64.0)
                else:
                    nc.scalar.activation(
                        mT[:, hc, h * 256:(h + 1) * 256], ps[:, 0, 0:256],
                        AF.Gelu, scale=1.0 / 64.0)

        with nc.named_scope("proj_ln2"):
            seg2s = {}
            proj_unit(0, 0, 4, 8)()
            proj_unit(0, 1, 4, 8)()
            warm(128, pool=psB)
            seg2s[0] = ln_seg1(x2[:, 0, :], name="h2")
            for tcq in range(nq):
                if tcq + 1 < nq:
                    proj_unit(tcq + 1, 0, 4, 8)()
                    proj_unit(tcq + 1, 1, 4, 8)()
                    seg2s[tcq + 1] = ln_seg1(x2[:, tcq + 1, :], name="h2")
                hq2 = ln_seg2(seg2s.pop(tcq), g2b, b2b, ln2buf, ln2_q,
                              name="h2")
                nc.scalar.dma_start_transpose(
                    out=h2Tb[:, :, tcq * 128:(tcq + 1) * 128], in_=hq2)
                nc.scalar.copy(h2T[:, :, tcq * 128:(tcq + 1) * 128],
                               h2Tb[:, :, tcq * 128:(tcq + 1) * 128])
                if has_b2:
                    nc.vector.tensor_tensor(out=x2[:, tcq, :], in0=x2[:, tcq, :],
                                            in1=bf2b, op=OP.add)
                if tcq == 2:
                    fc1_half(0)  # tokens 0-255: real PE work bridges LN2 tail
                else:
                    warm(512, pool=psB, reps=2)

        close_pool("wprojB")
        close_pool("o_pool")

        # wfc2 loads once proj psums + o_p freed
        wfc2p = open_pool("wfc2p", 1)
        wfc2_sb = wfc2p.tile([128, DFF // 128, D], BF16, name="wfc2")
        for g in range(8):
            eng = nc.sync if g % 2 == 0 else nc.gpsimd
            eng.dma_start(
                out=wfc2_sb[:, g * 4:(g + 1) * 4, :],
                in_=wfc2_d.rearrange("(c p) n -> p c n", p=128)[:, g * 4:(g + 1) * 4, :])

        # ---------------- fc1 second half + gelu ----------------
        with nc.named_scope("fc1"):
            fc1_half(1)

        # ---------------- fc2 + residual -> out ----------------
        outp = open_pool("outp", 2)
        with nc.named_scope("fc2"):
            for tcq in range(nq):
                ot = outp.tile([128, D], F32, name="ot")
                ps = psB.tile([128, 2, 512], F32, name="ps")
                for hc in range(DFF // 128):
                    # both output halves share one stationary mT load
                    for nn in range(2):
                        nc.tensor.matmul(
                            ps[:, nn, :], mT[:, hc, tcq * 128:(tcq + 1) * 128],
                            wfc2_sb[:, hc, nn * 512:(nn + 1) * 512],
                            start=(hc == 0), stop=(hc == DFF // 128 - 1))
                for nn in range(2):
                    nc.vector.tensor_tensor(out=ot[:, nn * 512:(nn + 1) * 512],
                                            in0=ps[:, nn, :],
                                            in1=x2[:, tcq, nn * 512:(nn + 1) * 512],
                                            op=OP.add)
                nc.sync.dma_start(out=out_d[tcq * 128:(tcq + 1) * 128, :], in_=ot)

        close_pool("outp")
        close_pool("wfc2p")
        close_pool("ln2_q")
        close_pool("ln2buf")
        close_pool("h2bp")
        close_pool("mlp")
        close_pool("mlpw")
        close_pool("psB")
        close_pool("psA")
        close_pool("resid")
        close_pool("small")
        close_pool("consts")

    nc.finalize()
    return nc


_NC_CACHE = {}


def _get_nc(Tq, Tkv, apply_gb=True, has_bp=False, has_b2=False, has_b1=False):
    key = (Tq, Tkv, apply_gb, has_bp, has_b2, has_b1)
    if key not in _NC_CACHE:
        _NC_CACHE[key] = build_nc(Tq, Tkv, apply_gb, has_bp, has_b2, has_b1)
    return _NC_CACHE[key]


def make_in_maps(x, ln1_g, ln1_b, ln2_g, ln2_b, w_qkv, w_proj, b_proj,
                 w_fc1, b_fc1, w_fc2, b_fc2, n_cores=8):
    x = np.asarray(x, np.float32)
    B, S, _ = x.shape
    half = S // 2
    shared = {
        "w_qkv": np.ascontiguousarray(np.asarray(w_qkv, np.float32)),
        "w_proj": np.ascontiguousarray(np.asarray(w_proj, np.float32)),
        "b_proj": np.asarray(b_proj, np.float32),
        "w_fc1": np.ascontiguousarray(
            (np.asarray(w_fc1, np.float32) * 64.0)
            .reshape(4, 2, 128, 4096 // 128, 128).transpose(3, 2, 0, 1, 4)
        ).astype(ml_dtypes.float8_e4m3),
        "b_fc1": np.asarray(b_fc1, np.float32),
        "w_fc2": np.ascontiguousarray(np.asarray(w_fc2, np.float32).astype(ml_dtypes.bfloat16)),
        "b_fc2": np.asarray(b_fc2, np.float32),
        "ln1_g": np.asarray(ln1_g, np.float32),
        "ln1_b": np.asarray(ln1_b, np.float32),
        "ln2_g": np.asarray(ln2_g, np.float32),
        "ln2_b": np.asarray(ln2_b, np.float32),
    }
    in_maps = []
    for c in range(n_cores):
        b, h = c // 2, c % 2
        xr = np.concatenate([x[b, h * half:(h + 1) * half],
                             x[b, (1 - h) * half:(2 - h) * half]], axis=0)
        in_maps.append({"x": np.ascontiguousarray(xr), **shared})
    return in_maps


def kernel(x, ln1_g, ln1_b, ln2_g, ln2_b, w_qkv, w_proj, b_proj,
           w_fc1, b_fc1, w_fc2, b_fc2, num_heads=16, block_size=16):
    x = np.asarray(x, np.float32)
    B, S, Dm = x.shape
    half = S // 2
    trivial_gb = (np.all(np.asarray(ln1_g) == 1) and np.all(np.asarray(ln2_g) == 1)
                  and np.all(np.asarray(ln1_b) == 0) and np.all(np.asarray(ln2_b) == 0))
    has_bp = not np.all(np.asarray(b_proj) == 0)
    has_b2 = not np.all(np.asarray(b_fc2) == 0)
    has_b1 = not np.all(np.asarray(b_fc1) == 0)
    nc = _get_nc(half, S, apply_gb=not trivial_gb, has_bp=has_bp,
                 has_b2=has_b2, has_b1=has_b1)
    in_maps = make_in_maps(x, ln1_g, ln1_b, ln2_g, ln2_b, w_qkv, w_proj, b_proj,
                           w_fc1, b_fc1, w_fc2, b_fc2)
    res = bass_utils.run_bass_kernel_spmd(nc, in_maps, core_ids=list(range(8)))
    out = np.empty((B, S, Dm), np.float32)
    for c in range(8):
        b, h = c // 2, c % 2
        out[b, h * half:(h + 1) * half] = res.results[c]["out"]
    return out


# revision 46
# speedup vs baseline: 1.0183x; 1.0183x over previous
"""Trainium2 Bass kernel for nn_Block_59983513256170 (dense transformer block).

Sharding: 8 cores = (batch 4) x (sequence halves 2). Each core computes the
block for 512 query tokens of one batch element, redundantly computing
LN1+quant and K/V over that batch element's full 1024-token sequence so no
cross-core communication is needed. Host rotates each core's token order so
its own 512 tokens are always rows [0:512] (attention is permutation
invariant over keys), letting all cores run one identical SPMD program.

Precision: attention branch (qkv/scores/AV/proj) runs on fp32r matmuls
(~12 mantissa bits at bf16 speed) because its output feeds the second
block-FP quantization, where noise flips quantization bins. The quantized
h1/h2 values (4-bit mantissa) are exact in bf16/fp8: transposes run through
the DMA XBAR in bf16, h2 feeds fc1 as fp8 (DoubleRow), fc2 runs in bf16.
LN, BFP quant, softmax arithmetic and residuals are fp32.

Schedule highlights (all engine-balanced + software-pipelined):
 - LN1: bn-stats of tile t+1 emitted before tile t's apply+quant (DVE never
   ping-pongs on ACT); chain split DVE/GpSimd/ACT; token->feature
   transposes via DMA XBAR (bf16 staging ring + ACT convert to f32r);
   v-units per tile and q/k units fill the PE during the LN tail.
 - Attention: the two heads of a pair run as concurrent row-tiled
   64-partition QK matmuls into one 2-bank psum, exp is fused over both
   banks in a single ACTIVATE, AV trails QK by one key chunk, and
   background PE work (remaining k/v units, then proj partial sums for
   heads 0-7) fills the exp-latency gaps so HAM stays at full clock.
 - Softmax normalization: fused psum*recip multiply (scalar_tensor_tensor)
   with a gpsimd partition-broadcast of the reciprocal row.
 - Back half: proj finishes (heads 8-15) skewed with LN2; fc1 (fp8
   DoubleRow) runs in token halves, the first bridging the LN2 tail so the
   PE clock never drops; fc2 shares one stationary load per output pair;
   residual adds on DVE.
"""

import sys

sys.path.insert(0, "/opt/trn_rl_repo")

import numpy as np
import ml_dtypes

import concourse.bass as bass
import concourse.bacc as bacc
import concourse.tile as tile
import concourse.mybir as mybir
from concourse import bass_utils

F32 = mybir.dt.float32
F32R = mybir.dt.float32r
BF16 = mybir.dt.bfloat16
F8 = mybir.dt.float8e4
DRM = mybir.MatmulPerfMode.DoubleRow
AF = mybir.ActivationFunctionType
OP = mybir.AluOpType

D = 1024
H = 16
DH = 64
DFF = 4096
LN_EPS = 1e-6


def bcast16(t):
    """View a [128, nb] tile as [128, nb, 16] with the last dim broadcast."""
    ap = [list(x) for x in t.ap]
    return bass.AP(tensor=t.tensor, offset=t.offset, ap=ap + [[0, 16]])


def pbcast(t, n):
    """View a [1, ...] tile as [n, ...] with the partition dim broadcast
    (for DMA replication)."""
    ap = [list(x) for x in t.ap]
    return bass.AP(tensor=t.tensor, offset=t.offset, ap=[[0, n]] + ap[1:])


def build_nc(Tq, Tkv, apply_gb=True, has_bp=False, has_b2=False,
             has_b1=False):
    """Build the per-core Bass program. Tq = own query tokens, Tkv = full
    sequence tokens of this core's batch element (own tokens first)."""
    nq = Tq // 128   # query token tiles (4)
    nk = Tkv // 128  # kv token tiles (8)
    npair = H // 2   # head pairs (8)
    nc = bacc.Bacc("TRN2", target_bir_lowering=False, debug=False)

    x_d = nc.dram_tensor("x", [Tkv, D], F32, kind="ExternalInput").ap()
    wqkv_d = nc.dram_tensor("w_qkv", [D, 3 * D], F32R, kind="ExternalInput").ap()
    wproj_d = nc.dram_tensor("w_proj", [D, D], F32R, kind="ExternalInput").ap()
    bproj_d = nc.dram_tensor("b_proj", [D], F32, kind="ExternalInput").ap()
    wfc1_d = nc.dram_tensor("w_fc1", [DFF // 128, 128, 4, 2, 128], F8,
                            kind="ExternalInput").ap()
    bfc1_d = nc.dram_tensor("b_fc1", [DFF], F32, kind="ExternalInput").ap()
    wfc2_d = nc.dram_tensor("w_fc2", [DFF, D], BF16, kind="ExternalInput").ap()
    bfc2_d = nc.dram_tensor("b_fc2", [D], F32, kind="ExternalInput").ap()
    g1_d = nc.dram_tensor("ln1_g", [D], F32, kind="ExternalInput").ap()
    b1_d = nc.dram_tensor("ln1_b", [D], F32, kind="ExternalInput").ap()
    g2_d = nc.dram_tensor("ln2_g", [D], F32, kind="ExternalInput").ap()
    b2_d = nc.dram_tensor("ln2_b", [D], F32, kind="ExternalInput").ap()
    out_d = nc.dram_tensor("out", [Tq, D], F32, kind="ExternalOutput").ap()

    def vec_bcast(pool, dram_vec, name, dtype=F32):
        """DRAM [D] vector -> SBUF [128, D] broadcast tile."""
        t = pool.tile([128, dram_vec.shape[0]], dtype, name=name)
        src = bass.AP(tensor=dram_vec.tensor, offset=dram_vec.offset,
                      ap=[[0, 128]] + [list(x) for x in dram_vec.ap])
        eng = nc.gpsimd if dtype != dram_vec.dtype else nc.sync
        eng.dma_start(out=t, in_=src)
        return t

    with tile.TileContext(nc) as tc:
        _cms = {}

        def open_pool(name, bufs, space="SBUF", side="left"):
            cm = tc.tile_pool(name=name, bufs=bufs, space=space, side=side)
            _cms[name] = cm
            return cm.__enter__()

        def close_pool(name):
            _cms.pop(name).__exit__(None, None, None)

        # non-default variants carry extra broadcast consts; shrink rings
        tight = apply_gb or has_bp or has_b2 or has_b1
        # ---- pool stacks (LIFO per side) ----
        consts = open_pool("consts", 1)
        small = open_pool("small", 2)
        resid = open_pool("resid", 1)
        attn_big = open_pool("attn_big", 1)
        h1fmp = open_pool("h1fmp", 1)
        wk_pool = open_pool("wk", 1)
        wv_pool = open_pool("wv", 1)
        h1t_pool = open_pool("h1t", 2 if tight else 3)
        lnx = open_pool("lnx", 1 if tight else 2)  # x tiles 4..7
        lnh = open_pool("lnh", 2)  # LN ht scratch
        ln_q = open_pool("ln_q", 1 if tight else 2)
        wq_pool = open_pool("wq", 1 if tight else 2)
        # psum
        psA = open_pool("psA", 2, space="PSUM")
        warm_ps = open_pool("warm_ps", 1, space="PSUM")

        eps_t = consts.tile([128, 1], F32, name="eps")
        nc.vector.memset(eps_t, LN_EPS)
        idw = consts.tile([128, 128], BF16, name="idw")
        nc.vector.memset(idw, 0.0)
        if not tight:
            idw512 = consts.tile([128, 512], BF16, name="idw512")
            nc.vector.memset(idw512, 0.0)
        else:
            idw512 = idw
        cdt = BF16 if tight else F32
        if apply_gb:
            g1b = vec_bcast(consts, g1_d, "g1b", cdt)
            b1b = vec_bcast(consts, b1_d, "b1b", cdt)
            g2b = vec_bcast(consts, g2_d, "g2b", cdt)
            b2b = vec_bcast(consts, b2_d, "b2b", cdt)
        else:
            g1b = b1b = g2b = b2b = None
        bpb = vec_bcast(consts, bproj_d, "bpb", cdt) if has_bp else None
        bf2b = vec_bcast(consts, bfc2_d, "bf2b", cdt) if has_b2 else None
        # b_fc1 as per-partition bias columns: [128, 32], [p, c] = b_fc1[c*128+p]
        bfc1_sb = consts.tile([128, DFF // 128], F32, name="bfc1")
        nc.sync.dma_start(out=bfc1_sb, in_=bfc1_d.rearrange("(c p) -> p c", p=128))

        def warm(n=64, pool=None, reps=1):
            """Small PE touch to keep/raise the HAM clock during PE-sparse
            stretches."""
            n = min(n, 128 if tight else 512)
            dp = (pool or warm_ps).tile([128, n], F32,
                                        name="ps" if pool else "warm")
            for r in range(reps):
                nc.tensor.matmul(dp, idw, idw512[:, 0:n],
                                 start=True, stop=True)

        # persistent tiles
        h1_fm = h1fmp.tile([128, 8, Tkv], F32R, name="h1fm")  # feature-major
        qT = attn_big.tile([128, npair, Tq], F32R, name="qT")
        kT = attn_big.tile([128, npair, Tkv], F32R, name="kT")
        v65 = attn_big.tile([128, H, nk, 65], F32R, name="v65")
        x2 = resid.tile([128, nq, D], F32, name="x2")
        # x2 doubles as the LN1 input for the own-token tiles 0..3
        for t in range(nq):
            nc.sync.dma_start(out=x2[:, t, :], in_=x_d[t * 128:(t + 1) * 128, :])

        # v65 ones plane (denominator column), one strided memset
        nc.gpsimd.memset(v65[:, :, :, 64:65].bitcast(F32), 1.0)

        def ln_seg1(xt, name="h"):
            """bn stats + sqrt: DVE then ACT, no downstream deps yet."""
            st = small.tile([128, 2, 6], F32, name=name + "st")
            nc.vector.bn_stats(out=st[:, 0, :], in_=xt[:, 0:512])
            nc.vector.bn_stats(out=st[:, 1, :], in_=xt[:, 512:1024])
            mv = small.tile([128, 2], F32, name=name + "mv")
            nc.vector.bn_aggr(out=mv, in_=st)
            rs = small.tile([128, 1], F32, name=name + "rs")
            nc.scalar.activation(rs, mv[:, 1:2], AF.Sqrt, bias=eps_t, scale=1.0)
            return xt, mv, rs

        def ln_seg2(seg, g_b, b_b, ht_pool, hq_out_pool, name="h"):
            """recip + apply (ACT Identity) + BFP quant (DVE/GpSimd)."""
            xt, mv, rs = seg
            rr = small.tile([128, 1], F32, name=name + "rr")
            nc.vector.reciprocal(rr, rs)
            nmr = small.tile([128, 1], F32, name=name + "nmr")
            nc.vector.tensor_tensor(out=nmr, in0=mv[:, 0:1], in1=rr, op=OP.mult)
            nc.vector.tensor_scalar_mul(nmr, nmr, -1.0)
            ht = ht_pool.tile([128, D], F32, name="lnb")
            nc.scalar.activation(ht, xt, AF.Identity, bias=nmr, scale=rr)
            if g_b is not None:
                nc.gpsimd.tensor_tensor(out=ht, in0=ht, in1=g_b, op=OP.mult)
                nc.gpsimd.tensor_tensor(out=ht, in0=ht, in1=b_b, op=OP.add)
            nb = D // 16
            amax = small.tile([128, nb], F32, name=name + "amax")
            nc.vector.tensor_reduce(amax, ht.rearrange("p (b k) -> p b k", k=16),
                                    axis=mybir.AxisListType.X, op=OP.max,
                                    apply_absolute_value=True)
            nc.vector.tensor_scalar(out=amax.bitcast(mybir.dt.uint32),
                                    in0=amax.bitcast(mybir.dt.uint32),
                                    scalar1=0xFF800000, scalar2=None,
                                    op0=OP.bitwise_and)
            cc = small.tile([128, nb], F32, name=name + "cc")
            nc.vector.tensor_scalar_mul(cc, amax, float(1.5 * 2 ** 20))
            hi = small.tile([128, nb], F32, name=name + "hi")
            nc.vector.tensor_scalar_mul(hi, amax, 7.0 / 8.0)
            lo = small.tile([128, nb], F32, name=name + "lo")
            nc.vector.tensor_scalar_mul(lo, amax, -7.0 / 8.0)
            h3 = ht.rearrange("p (b k) -> p b k", k=16)
            nc.gpsimd.tensor_tensor(out=h3, in0=h3, in1=bcast16(cc), op=OP.add)
            nc.gpsimd.tensor_tensor(out=h3, in0=h3, in1=bcast16(cc),
                                    op=OP.subtract)
            nc.vector.tensor_tensor(out=h3, in0=h3, in1=bcast16(hi), op=OP.min)
            hq = hq_out_pool.tile([128, D], BF16, name=name + "q")
            nc.vector.tensor_tensor(out=hq.rearrange("p (b k) -> p b k", k=16),
                                    in0=h3, in1=bcast16(lo), op=OP.max)
            return hq

        def layernorm_quant(xt, g_b, b_b, ht_pool, hq_out_pool, name="h"):
            return ln_seg2(ln_seg1(xt, name), g_b, b_b, ht_pool, hq_out_pool,
                           name)

        # ---- qkv unit emitters ----
        def q_unit(j, wq):
            """q for head pair j: psum bank <- 8 accum MMs, copy to qT (ACT)."""
            ps = psA.tile([128, Tq], F32, name="ps")
            for d in range(8):
                nc.tensor.matmul(ps, wq[:, d, :], h1_fm[:, d, 0:Tq],
                                 start=(d == 0), stop=(d == 7))
            nc.scalar.copy(qT[:, j, :], ps)

        def load_wk(j0, engs=None):
            wk = wk_pool.tile([128, 4, 8, 128], F32R, name="wk")
            for j in range(j0, j0 + 4):
                eng = (engs or (nc.gpsimd,))[j % len(engs or (nc.gpsimd,))]
                eng.dma_start(
                    out=wk[:, j - j0, :, :],
                    in_=wqkv_d[:, D + j * 128:D + (j + 1) * 128]
                    .rearrange("(a p) c -> p a c", p=128))
            return wk

        wk_state = {}

        def k_unit(j, th):
            """k for head pair j, token half th: 8 accum MMs + DVE copy."""
            wk = wk_state[j // 4]
            ps = psA.tile([128, 512], F32, name="ps")
            for d in range(8):
                nc.tensor.matmul(ps, wk[:, j % 4, d, :],
                                 h1_fm[:, d, th * 512:(th + 1) * 512],
                                 start=(d == 0), stop=(d == 7))
            nc.vector.tensor_copy(out=kT[:, j, th * 512:(th + 1) * 512], in_=ps)

        def v_unit(vc, tch, wv, copy_eng=None):
            """v for head half vc, token tile tch: 8 accum MMs + one copy
            into v65 (8 heads' 64-col slices)."""
            ps = psA.tile([128, 512], F32, name="ps")
            for d in range(8):
                nc.tensor.matmul(ps, h1_fm[:, d, tch * 128:(tch + 1) * 128],
                                 wv[:, d, :], start=(d == 0), stop=(d == 7))
            dst = v65[:, vc * 8:(vc + 1) * 8, tch, 0:64]
            srcp = ps.rearrange("p (h e) -> p h e", e=64)
            if copy_eng is nc.scalar:
                nc.scalar.copy(dst, srcp)
            else:
                nc.vector.tensor_copy(out=dst, in_=srcp)

        # ---------------- phase A: LN1 + qkv interleaved ----------------
        wq_tiles = {}
        with nc.named_scope("ln1_qkv"):
            wv0 = wv_pool.tile([128, 8, 512], F32R, name="wv")
            for d in range(8):
                nc.gpsimd.dma_start(out=wv0[:, d, :],
                                    in_=wqkv_d[d * 128:(d + 1) * 128, 2048:2560])
            wk_state[0] = load_wk(0)

            def load_wq(j):
                wq = wq_pool.tile([128, 8, 128], F32R, name="wq")
                nc.sync.dma_start(
                    out=wq,
                    in_=wqkv_d[:, j * 128:(j + 1) * 128]
                    .rearrange("(a p) c -> p a c", p=128))
                wq_tiles[j] = wq

            # preload x tiles 4..7 (ring 2: 6/7 issued mid-loop)
            xts = {}
            for t in (4, 5):
                xts[t] = lnx.tile([128, D], F32, name="xt")
                nc.sync.dma_start(out=xts[t],
                                  in_=x_d[t * 128:(t + 1) * 128, :])
            segs = {0: ln_seg1(x2[:, 0, :], name="h1")}
            for t in range(nk):
                # stats of the next tile first: keeps DVE fed while ACT/GpSimd
                # work on this tile's apply+quant
                if t + 1 < nk:
                    if t + 1 < nq:
                        xt1 = x2[:, t + 1, :]
                    else:
                        xt1 = xts.pop(t + 1)
                    segs[t + 1] = ln_seg1(xt1, name="h1")
                hq = ln_seg2(segs.pop(t), g1b, b1b, lnh, ln_q, name="h1")
                tr = h1t_pool.tile([128, 8, 128], BF16, name="h1t")
                nc.scalar.dma_start_transpose(out=tr, in_=hq)
                nc.scalar.copy(h1_fm[:, :, t * 128:(t + 1) * 128], tr)
                warm(512, reps=2)
                v_unit(0, t, wv0, copy_eng=nc.scalar)
                if t == 3:
                    load_wq(0)
                    load_wq(1)
                elif t in (4, 5, 6):
                    if t in (4, 5):
                        xts[t + 2] = lnx.tile([128, D], F32, name="xt")
                        nc.sync.dma_start(out=xts[t + 2],
                                          in_=x_d[(t + 2) * 128:(t + 3) * 128, :])
                    j0 = (t - 4) * 2
                    for j in (j0, j0 + 1):
                        q_unit(j, wq_tiles.pop(j))
                        if j + 2 < npair:
                            load_wq(j + 2)
                elif t == 7:
                    for j in (6, 7):
                        q_unit(j, wq_tiles.pop(j))
                    for j in range(4):
                        k_unit(j, 0)
            # k(th1) for j=0,1 so attention can start
            k_unit(0, 1)
            k_unit(1, 1)
        close_pool("wq")
        close_pool("ln_q")
        close_pool("lnh")
        close_pool("lnx")
        close_pool("h1t")
        close_pool("warm_ps")

        if has_bp:
            for tcq in range(nq):
                nc.vector.tensor_tensor(out=x2[:, tcq, :], in0=x2[:, tcq, :],
                                        in1=bpb, op=OP.add)

        # ---------------- attention + background work ----------------
        ps_pair = open_pool("ps_pair", 2, space="PSUM")  # [128,2,512] F32
        ps_o = open_pool("ps_o", 1, space="PSUM")        # [65,2,512] F32
        o_pool = open_pool("o_pool", 1, side="right")
        ee_pool = open_pool("ee_pool", 2 if tight else 3, side="right")
        nrm = open_pool("nrm", 1, side="right")
        o_p = o_pool.tile([128, npair, Tq], F32R, name="o_p")
        wproj_sb = [None, None]
        wfc1_sb_box = [None]
        bg_state = {}

        def bg_k(j, th=1):
            def f():
                k_unit(j, th)
            return f

        def bg_k_halves(j, th):
            state = {}
            wkref = wk_state

            def f1():
                wk = wkref[j // 4]
                ps = psA.tile([128, 512], F32, name="ps")
                for d in range(4):
                    nc.tensor.matmul(ps, wk[:, j % 4, d, :],
                                     h1_fm[:, d, th * 512:(th + 1) * 512],
                                     start=(d == 0), stop=False)
                state["ps"] = ps

            def f2():
                wk = wkref[j // 4]
                ps = state["ps"]
                for d in range(4, 8):
                    nc.tensor.matmul(ps, wk[:, j % 4, d, :],
                                     h1_fm[:, d, th * 512:(th + 1) * 512],
                                     start=False, stop=(d == 7))
                nc.vector.tensor_copy(out=kT[:, j, th * 512:(th + 1) * 512],
                                      in_=ps)
            return f1, f2

        def bg_wk1_load():
            def f():
                wk_state[1] = load_wk(4, engs=(nc.sync, nc.gpsimd))
            return f

        def bg_wv1_load():
            def f():
                wv1 = wv_pool.tile([128, 8, 512], F32R, name="wv")
                for d in range(8):
                    eng = nc.sync if d % 2 == 0 else nc.gpsimd
                    eng.dma_start(
                        out=wv1[:, d, :],
                        in_=wqkv_d[d * 128:(d + 1) * 128, 2560:3072])
                bg_state["wv1"] = wv1
            return f

        def bg_v1(tch):
            def f():
                v_unit(1, tch, bg_state["wv1"])
            return f

        def bg_close_a():
            def f():
                close_pool("wv")
                close_pool("wk")
                close_pool("h1fmp")
                wpp = open_pool("wprojA", 1)  # left, above attn_big
                wp = wpp.tile([128, 4, D], F32R, name="wprojA")
                for d in range(4):
                    eng = nc.sync if d % 2 == 0 else nc.gpsimd
                    eng.dma_start(out=wp[:, d, :],
                                  in_=wproj_d[d * 128:(d + 1) * 128, :])
                wproj_sb[0] = wp
            return f

        def proj_unit(tcq, nn, j0, j1):
            """Partial proj over head pairs [j0,j1) for (token tile, half),
            accumulated in psum then added into x2 on DVE."""
            def f():
                ps = psA.tile([128, 512], F32, name="ps")
                wp = wproj_sb[0] if j0 == 0 else wproj_sb[1]
                for j in range(j0, j1):
                    nc.tensor.matmul(ps, o_p[:, j, tcq * 128:(tcq + 1) * 128],
                                     wp[:, j - j0, nn * 512:(nn + 1) * 512],
                                     start=(j == j0), stop=(j == j1 - 1))
                nc.vector.tensor_tensor(
                    out=x2[:, tcq, nn * 512:(nn + 1) * 512], in0=ps,
                    in1=x2[:, tcq, nn * 512:(nn + 1) * 512], op=OP.add)
            return f

        k2 = bg_k_halves(2, 1)
        k3 = bg_k_halves(3, 1)
        k4a, k4b = bg_k_halves(4, 0), bg_k_halves(4, 1)
        k5a, k5b = bg_k_halves(5, 0), bg_k_halves(5, 1)
        k6a, k6b = bg_k_halves(6, 0), bg_k_halves(6, 1)
        k7a, k7b = bg_k_halves(7, 0), bg_k_halves(7, 1)
        bg_sched = {
            0: [k2[0], k2[1], k3[0], k3[1], bg_wk1_load(), bg_wv1_load()],
            1: [k4a[0], k4a[1], k4b[0], k4b[1], bg_v1(0), bg_v1(1)],
            2: [k5a[0], k5a[1], k5b[0], k5b[1], bg_v1(2), bg_v1(3)],
            3: [k6a[0], k6a[1], k6b[0], k6b[1], bg_v1(4), bg_v1(5)],
            4: [k7a[0], k7a[1], bg_v1(6), bg_v1(7), k7b[0], k7b[1]],
            5: [bg_close_a(), proj_unit(0, 0, 0, 4), proj_unit(0, 1, 0, 4),
                proj_unit(1, 0, 0, 4), proj_unit(1, 1, 0, 4)],
            6: [proj_unit(2, 0, 0, 4), proj_unit(2, 1, 0, 4),
                proj_unit(3, 0, 0, 4)],
            7: [proj_unit(3, 1, 0, 4)],
        }

        with nc.named_scope("attn"):
            for j in range(npair):
                units = list(bg_sched[j])
                pos = 0

                def pop_bg():
                    nonlocal pos
                    if pos < len(units):
                        units[pos]()
                        pos += 1
                    else:
                        warm(512, pool=psA, reps=2)

                po = ps_o.tile([65, 2, Tq], F32, name="pso")
                ees = {}
                for kc in range(nk):
                    pp = ps_pair.tile([128, 2, Tq], F32, name="pp")
                    for ab in range(2):
                        nc.tensor.matmul(
                            pp[:, ab, :],
                            kT[ab * 64:(ab + 1) * 64, j,
                               kc * 128:(kc + 1) * 128],
                            qT[ab * 64:(ab + 1) * 64, j, :],
                            start=True, stop=True)
                    if kc > 0:
                        ee = ees.pop(kc - 1)
                        for ab in range(2):
                            nc.tensor.matmul(po[:, ab, :],
                                             v65[:, 2 * j + ab, kc - 1, :],
                                             ee[:, ab, :],
                                             start=(kc - 1 == 0), stop=False)
                    ee = ee_pool.tile([128, 2, Tq], F32R, name="ee")
                    nc.scalar.activation(ee, pp, AF.Exp, scale=0.125)
                    ees[kc] = ee
                    if kc >= 1:
                        pop_bg()
                ee = ees.pop(nk - 1)
                for ab in range(2):
                    nc.tensor.matmul(po[:, ab, :], v65[:, 2 * j + ab, nk - 1, :],
                                     ee[:, ab, :], start=False, stop=True)
                while pos < len(units):
                    units[pos]()
                    pos += 1
                # normalize: o_p = po[0:64] * (1/po[64]) broadcast
                row = nrm.tile([1, 2, Tq], F32, name="row")
                nc.vector.tensor_copy(out=row, in_=po[64:65, :, :])
                rrow = nrm.tile([1, 2, Tq], F32, name="rrow")
                nc.vector.reciprocal_approx_fast(rrow, row)
                r64 = nrm.tile([64, 2, Tq], F32, name="r64")
                nc.gpsimd.partition_broadcast(r64, rrow)
                for ab in range(2):
                    nc.vector.scalar_tensor_tensor(
                        out=o_p[ab * 64:(ab + 1) * 64, j, :],
                        in0=po[0:64, ab, :], scalar=1.0, in1=r64[:, ab, :],
                        op0=OP.mult, op1=OP.mult)

        close_pool("nrm")
        close_pool("ee_pool")
        close_pool("ps_o")
        close_pool("ps_pair")
        close_pool("wprojA")
        close_pool("attn_big")

        # ---------------- phase B: proj(B) + LN2 + MLP ----------------
        psB = open_pool("psB", 2, space="PSUM")
        # wproj rows for heads 8-15 (right side: closes before o_pool)
        wpbp = open_pool("wprojB", 1, side="right")
        wpb = wpbp.tile([128, 4, D], F32R, name="wprojB")
        for d in range(4):
            nc.sync.dma_start(out=wpb[:, d, :],
                              in_=wproj_d[(4 + d) * 128:(5 + d) * 128, :])
        wproj_sb[1] = wpb
        # fc1 weights (left): DMA starts now, consumed from fc1 onwards
        mlpw = open_pool("mlpw", 1)
        wfc1_sb = mlpw.tile([128, DFF // 128, 4, 2, 128], F8, name="wfc1")
        for hc in range(DFF // 128):
            eng = nc.sync if hc % 2 == 0 else nc.gpsimd
            eng.dma_start(out=wfc1_sb[:, hc], in_=wfc1_d[hc])
        mlp = open_pool("mlp", 1)
        mT = mlp.tile([128, DFF // 128, Tq], BF16, name="mT")
        h2bp = open_pool("h2bp", 1)
        h2Tb = h2bp.tile([128, 8, Tq], BF16, name="h2Tb")
        h2T = h2bp.tile([128, 8, Tq], F8, name="h2T")
        ln2buf = open_pool("ln2buf", 2)
        ln2_q = open_pool("ln2_q", 2)

        def fc1_half(h):
            for hc in range(DFF // 128):
                ps = psB.tile([128, 2, 512], F32, name="ps")
                for d2 in range(4):
                    nc.tensor.matmul(
                        ps[:, 0, 0:256], wfc1_sb[:, hc, d2, :, :],
                        h2T[:, 2 * d2:2 * d2 + 2, h * 256:(h + 1) * 256],
                        start=(d2 == 0), stop=(d2 == 3), perf_mode=DRM)
                if has_b1:
                    nc.scalar.activation(
                        mT[:, hc, h * 256:(h + 1) * 256], ps[:, 0, 0:256],
                        AF.Gelu, bias=bfc1_sb[:, hc:hc + 1], scale=1.0 / 64.0)
                else:
                    nc.scalar.activation(
                        mT[:, hc, h * 256:(h + 1) * 256], ps[:, 0, 0:256],
                        AF.Gelu, scale=1.0 / 64.0)

        with nc.named_scope("proj_ln2"):
            seg2s = {}
            proj_unit(0, 0, 4, 8)()
            proj_unit(0, 1, 4, 8)()
            warm(128, pool=psB)
            seg2s[0] = ln_seg1(x2[:, 0, :], name="h2")
            for tcq in range(nq):
                if tcq + 1 < nq:
                    proj_unit(tcq + 1, 0, 4, 8)()
                    proj_unit(tcq + 1, 1, 4, 8)()
                    seg2s[tcq + 1] = ln_seg1(x2[:, tcq + 1, :], name="h2")
                hq2 = ln_seg2(seg2s.pop(tcq), g2b, b2b, ln2buf, ln2_q,
                              name="h2")
                nc.scalar.dma_start_transpose(
                    out=h2Tb[:, :, tcq * 128:(tcq + 1) * 128], in_=hq2)
                nc.scalar.copy(h2T[:, :, tcq * 128:(tcq + 1) * 128],
                               h2Tb[:, :, tcq * 128:(tcq + 1) * 128])
                if has_b2:
                    nc.vector.tensor_tensor(out=x2[:, tcq, :], in0=x2[:, tcq, :],
                                            in1=bf2b, op=OP.add)
                if tcq == 2:
                    fc1_half(0)  # tokens 0-255: real PE work bridges LN2 tail
                else:
                    warm(512, pool=psB, reps=2)

        close_pool("wprojB")
        close_pool("o_pool")

        # wfc2 loads once proj psums + o_p freed
        wfc2p = open_pool("wfc2p", 1)
        wfc2_sb = wfc2p.tile([128, DFF // 128, D], BF16, name="wfc2")
        for g in range(8):
            eng = nc.sync if g % 2 == 0 else nc.gpsimd
            eng.dma_start(
                out=wfc2_sb[:, g * 4:(g + 1) * 4, :],
                in_=wfc2_d.rearrange("(c p) n -> p c n", p=128)[:, g * 4:(g + 1) * 4, :])

        # ---------------- fc1 second half + gelu ----------------
        with nc.named_scope("fc1"):
            fc1_half(1)

        # ---------------- fc2 + residual -> out ----------------
        outp = open_pool("outp", 2)
        with nc.named_scope("fc2"):
            for tcq in range(nq):
                ot = outp.tile([128, D], F32, name="ot")
                ps = psB.tile([128, 2, 512], F32, name="ps")
                for hc in range(DFF // 128):
                    # both output halves share one stationary mT load
                    for nn in range(2):
                        nc.tensor.matmul(
                            ps[:, nn, :], mT[:, hc, tcq * 128:(tcq + 1) * 128],
                            wfc2_sb[:, hc, nn * 512:(nn + 1) * 512],
                            start=(hc == 0), stop=(hc == DFF // 128 - 1))
                for nn in range(2):
                    nc.vector.tensor_tensor(out=ot[:, nn * 512:(nn + 1) * 512],
                                            in0=ps[:, nn, :],
                                            in1=x2[:, tcq, nn * 512:(nn + 1) * 512],
                                            op=OP.add)
                nc.sync.dma_start(out=out_d[tcq * 128:(tcq + 1) * 128, :], in_=ot)

        close_pool("outp")
        close_pool("wfc2p")
        close_pool("ln2_q")
        close_pool("ln2buf")
        close_pool("h2bp")
        close_pool("mlp")
        close_pool("mlpw")
        close_pool("psB")
        close_pool("psA")
        close_pool("resid")
        close_pool("small")
        close_pool("consts")

    nc.finalize()
    return nc


_NC_CACHE = {}


def _get_nc(Tq, Tkv, apply_gb=True, has_bp=False, has_b2=False, has_b1=False):
    key = (Tq, Tkv, apply_gb, has_bp, has_b2, has_b1)
    if key not in _NC_CACHE:
        _NC_CACHE[key] = build_nc(Tq, Tkv, apply_gb, has_bp, has_b2, has_b1)
    return _NC_CACHE[key]


def make_in_maps(x, ln1_g, ln1_b, ln2_g, ln2_b, w_qkv, w_proj, b_proj,
                 w_fc1, b_fc1, w_fc2, b_fc2, n_cores=8):
    x = np.asarray(x, np.float32)
    B, S, _ = x.shape
    half = S // 2
    shared = {
        "w_qkv": np.ascontiguousarray(np.asarray(w_qkv, np.float32)),
        "w_proj": np.ascontiguousarray(np.asarray(w_proj, np.float32)),
        "b_proj": np.asarray(b_proj, np.float32),
        "w_fc1": np.ascontiguousarray(
            (np.asarray(w_fc1, np.float32) * 64.0)
            .reshape(4, 2, 128, 4096 // 128, 128).transpose(3, 2, 0, 1, 4)
        ).astype(ml_dtypes.float8_e4m3),
        "b_fc1": np.asarray(b_fc1, np.float32),
        "w_fc2": np.ascontiguousarray(np.asarray(w_fc2, np.float32).astype(ml_dtypes.bfloat16)),
        "b_fc2": np.asarray(b_fc2, np.float32),
        "ln1_g": np.asarray(ln1_g, np.float32),
        "ln1_b": np.asarray(ln1_b, np.float32),
        "ln2_g": np.asarray(ln2_g, np.float32),
        "ln2_b": np.asarray(ln2_b, np.float32),
    }
    in_maps = []
    for c in range(n_cores):
        b, h = c // 2, c % 2
        xr = np.concatenate([x[b, h * half:(h + 1) * half],
                             x[b, (1 - h) * half:(2 - h) * half]], axis=0)
        in_maps.append({"x": np.ascontiguousarray(xr), **shared})
    return in_maps


def kernel(x, ln1_g, ln1_b, ln2_g, ln2_b, w_qkv, w_proj, b_proj,
           w_fc1, b_fc1, w_fc2, b_fc2, num_heads=16, block_size=16):
    x = np.asarray(x, np.float32)
    B, S, Dm = x.shape
    half = S // 2
    trivial_gb = (np.all(np.asarray(ln1_g) == 1) and np.all(np.asarray(ln2_g) == 1)
                  and np.all(np.asarray(ln1_b) == 0) and np.all(np.asarray(ln2_b) == 0))
    has_bp = not np.all(np.asarray(b_proj) == 0)
    has_b2 = not np.all(np.asarray(b_fc2) == 0)
    has_b1 = not np.all(np.asarray(b_fc1) == 0)
    nc = _get_nc(half, S, apply_gb=not trivial_gb, has_bp=has_bp,
                 has_b2=has_b2, has_b1=has_b1)
    in_maps = make_in_maps(x, ln1_g, ln1_b, ln2_g, ln2_b, w_qkv, w_proj, b_proj,
                           w_fc1, b_fc1, w_fc2, b_fc2)
    res = bass_utils.run_bass_kernel_spmd(nc, in_maps, core_ids=list(range(8)))
    out = np.empty((B, S, Dm), np.float32)
    for c in range(8):
        b, h = c // 2, c % 2
        out[b, h * half:(h + 1) * half] = res.results[c]["out"]
    return out
